# revision 24
# baseline (speedup 1.0000x reference)
"""Multi-head self-attention (qk-l2-normalized) TRN2 Bass kernel.

Reference computation (T=4096, D=2048, H=16, HD=128):
    qkv = x @ W_qkv ; q,k,v = split(qkv)
    per head: qn = l2norm(q), kn = l2norm(k)
              attn = softmax(qn @ kn.T * HD**-0.5 + mask)
              o = attn @ v
    out = concat_heads(o) @ W_out

Sharding: tensor-parallel over heads.  Core c owns heads {2c, 2c+1}:
W_qkv column slices + W_out row slices.  Each core computes a partial
(T, D) output; the host sums the 8 partials (the "all-reduce").

Device algorithm per core (v2 — restructured for engine balance):
  - q/k projections run in fp8 (e4m3) with DoubleRow matmuls (contraction
    256 per instruction -> ~1.5x PE throughput).  The fp8 quantization
    error only perturbs the softmax logits (q/k are l2-normalized right
    after), contributing ~3e-4 relative error to the output.  W_q/W_k are
    pre-scaled by 32 on the host so their values sit in e4m3's normal
    range; the l2 normalization cancels the scale exactly.
  - V projection stays fp16 (v errors pass straight to the output).
  - Stage A: K projection + normalize, V projection, for all 8 token
    tiles, with DMA staged kc-chunk-wise so the first matmul starts
    ~4us in.  Q projection for tile 0 at the end.
  - Stage B: per token tile: attention for head 0, then the q-projection
    "stripe" for the next tile (independent PE work that fills the PE
    bubble while DVE finishes the softmax-denominator tree), then the
    Z path, then head 1 + its stripe, then the output projection.
  - softmax denominator: all-fp16 pair tree (DVE 2x mode), with the t1
    adds offloaded to the otherwise-idle GPSIMD engine; 1/Z via DVE
    reciprocal (exact) instead of the Ln/Exp trick (frees ACT cycles
    for the exp stream, which is the #2 engine load).
  - row-scale broadcasts (1/|q|, 1/Z) stay as ones-column rank-1 PE
    matmuls (cheap, and cross-partition broadcast has no other fast path
    on this toolchain).
"""

import os
import sys

import numpy as np

if "/opt/trn_rl_repo" not in sys.path:
    sys.path.insert(0, "/opt/trn_rl_repo")

T, D, H, NCORES = 4096, 2048, 16, 8
HD = D // H            # 128 head dim
HPC = H // NCORES      # 2 heads per core
DH = HPC * HD          # 256 local head columns
EPS = 1e-12
SCALE = HD ** -0.5
W8SCALE = 32.0         # host pre-scale for fp8 q/k weights

_PROG_CACHE = {}


def _split_drain_tc(nc, tile):
    """TileContext that never emits more than one semaphore wait per inst.

    This walrus build encodes only a single sync wait per instruction
    ("Too many sync wait commands" otherwise).  Two fixes:
    - interior instructions: after Tile's sem assignment, excess waits are
      moved onto same-engine InstNoOps inserted immediately before the
      instruction (engines execute their stream in order, so semantics are
      identical);
    - the kernel-tail drain: emit one wait-carrying SP nop per logical proc
      instead of attaching the whole global clock to the drain.
    """
    import bass_rust
    import concourse.mybir as mybir
    from concourse.vector_clock import ScopedClock, VectorClock

    MAXW = 1

    class SplitWaitTC(tile.TileContext):
        def _lower_ordered_insts(self, ordered):
            for bb_name, insts in ordered.items():
                new = []
                for inst in insts:
                    si = None
                    try:
                        si = inst.sync_info
                    except Exception:
                        pass
                    if si is not None and len(si.on_wait) > MAXW:
                        waits = list(si.on_wait)
                        keep, extra = waits[-MAXW:], waits[:-MAXW]
                        for i, w in enumerate(extra):
                            new.append(mybir.InstNoOp(
                                name=f"{inst.name}ws{i}",
                                engine=inst.engine,
                                bass_nofuse=True,
                                sync_info=bass_rust.SyncInfo(
                                    on_wait=[w], on_update=[]),
                            ))
                        inst.sync_info = bass_rust.SyncInfo(
                            on_wait=keep, on_update=list(si.on_update))
                    new.append(inst)
                ordered[bb_name] = new
            return super()._lower_ordered_insts(ordered)

        def _drain_and_barrier(self, tick_clock, wait_clock):
            ticks = eval(
                str(tick_clock.global_clock).replace("VectorClock(", "").rstrip(")"))
            for p, tk in enumerate(ticks):
                if tk > 0:
                    sub = VectorClock()
                    sub.require_at_least(p, tk)
                    nop = self.nc.sync.nop(nofuse=True)
                    wait_clock.add_sem_waits(nop.ins, ScopedClock({None: sub}))
            self.nc.sync.drain()
            self.nc.all_engine_barrier()
            assert self.sems is not None
            popped = self.nc._tile_sem_poison_stack.pop()
            assert popped is self._sem_poison
            self.nc.clear_and_free_semaphores(list(self.sems.allocated().values()))
            self.nc.all_engine_barrier()

    return SplitWaitTC(nc)


def build_program(t=T, with_mask=False):
    """Build the single-core Bass/Tile program (same program on all cores)."""
    import concourse.bass as bass
    import concourse.mybir as mybir
    import concourse.tile as tile

    dt = mybir.dt
    f32, f16, f8 = dt.float32, dt.float16, dt.float8e4
    AF = mybir.ActivationFunctionType
    DR = mybir.MatmulPerfMode.DoubleRow

    KC = D // 128          # 16 contraction chunks for projections
    KP = KC // 2           # 8 fp8 DoubleRow contraction pairs
    TTS = 512              # token tile size (free dim of most matmuls)
    NTT = t // TTS         # number of token tiles
    NJC = t // 128         # number of key chunks
    NST = TTS // 128       # 128-token subtiles per token tile

    nc = bass.Bass(trn_type="TRN2")
    xT_d = nc.dram_tensor("xT", (D, t), f16, kind="ExternalInput")
    x8_d = nc.dram_tensor("x8", (D, t), f8, kind="ExternalInput")
    wq_d = nc.dram_tensor("wq8", (D, DH), f8, kind="ExternalInput")
    wk_d = nc.dram_tensor("wk8", (D, DH), f8, kind="ExternalInput")
    wv_d = nc.dram_tensor("wv", (D, DH), f16, kind="ExternalInput")
    wo_d = nc.dram_tensor("wo", (DH, D), f16, kind="ExternalInput")
    if with_mask:
        mT_d = nc.dram_tensor("maskT", (t, t), f16, kind="ExternalInput")
    y_d = nc.dram_tensor("y", (t, D), f32, kind="ExternalOutput")

    xT_t = xT_d[:].rearrange("(kc p) t -> p kc t", p=128)   # (128, KC, t)
    x8_t = x8_d[:].rearrange("(kc p) t -> p kc t", p=128)
    wq_r = wq_d[:].rearrange("(kc p) m -> p kc m", p=128)
    wk_r = wk_d[:].rearrange("(kc p) m -> p kc m", p=128)
    wv_r = wv_d[:].rearrange("(kc p) m -> p kc m", p=128)
    wo_r = wo_d[:].rearrange("(h p) n -> p h n", p=128)

    with _split_drain_tc(nc, tile) as tc:
        with (
            tc.tile_pool(name="consts", bufs=1) as cpool,
            tc.tile_pool(name="wts", bufs=1) as wpool,
            tc.tile_pool(name="big", bufs=1) as bigpool,
            tc.tile_pool(name="xv", bufs=2) as xvpool,
            tc.tile_pool(name="x8p", bufs=2) as x8pool,
            tc.tile_pool(name="qn", bufs=3) as qnpool,
            tc.tile_pool(name="work", bufs=2) as work,
            tc.tile_pool(name="rows", bufs=3) as rows,
            tc.tile_pool(name="ps", bufs=1, space="PSUM") as psum,
        ):
            # PSUM budget (8 banks):
            #   st: (128,1024) 2-bank x2 = 4  [S^T pairs]
            #   ot: (128,512)  1-bank x2 = 2  [attn@v accumulators]
            #   sp: (128,512)  1-bank x2 = 2  [proj accums, outproj, aux rows]

            # ---- constants -------------------------------------------------
            ones_col = cpool.tile([1, 128], f16)    # lhsT for row->(128,·) bcast
            nc.vector.memset(ones_col[:], 1.0)
            ones_red = cpool.tile([128, 1], f16)    # lhsT for partition-sum
            nc.vector.memset(ones_red[:], 1.0)
            ln_scale_c = cpool.tile([1, 1], f32)    # bias: ln(SCALE) for rk
            nc.vector.memset(ln_scale_c[:], float(np.log(SCALE)))

            # ---- persistent activations -----------------------------------
            knt = bigpool.tile([128, HPC, t], f16, name="knt")
            vsb = bigpool.tile([128, NJC, DH], f16, name="vsb")

            # ---- weights ---------------------------------------------------
            wk8_sb = wpool.tile([128, KC, DH], f8)
            wq8_sb = wpool.tile([128, KC, DH], f8)
            wv_sb = wpool.tile([128, KC, DH], f16)
            wo_sb = wpool.tile([128, HPC, D], f16)

            # ---- staged DMA: wk8 + x8[0] first (kc-pair interleaved so
            # the first DoubleRow matmul can start after ~200KB) ----
            x8a = {}
            x8a[0] = x8pool.tile([128, KC, TTS], f8, tag="x8a", bufs=2, name="x8a")
            for kp in range(KP):
                ks = slice(2 * kp, 2 * kp + 2)
                nc.sync.dma_start(wk8_sb[:, ks, :], wk_r[:, ks, :])
                nc.sync.dma_start(x8a[0][:, ks, :], x8_t[:, ks, 0:TTS])
            for kh in range(4):
                ks = slice(kh * 4, (kh + 1) * 4)
                nc.sync.dma_start(wv_sb[:, ks, :], wv_r[:, ks, :])
            xv = {}
            xv[0] = xvpool.tile([128, KC, TTS], f16, tag="xv", bufs=2, name="xv")
            for kh in range(4):
                ks = slice(kh * 4, (kh + 1) * 4)
                nc.sync.dma_start(xv[0][:, ks, :], xT_t[:, ks, 0:TTS])
            for kh in range(4):
                ks = slice(kh * 4, (kh + 1) * 4)
                nc.sync.dma_start(wq8_sb[:, ks, :], wq_r[:, ks, :])


            qn_tiles = {}

            def emit_proj8(w_sb, x8t, hh, name, tag="sp"):
                """fp8 DoubleRow projection of one head for one token tile.
                Returns the fp32 PSUM accumulator (128=d_head, TTS)."""
                pj = psum.tile([128, TTS], f32, name=name, tag=tag, bufs=2)
                hsl = slice(hh * HD, (hh + 1) * HD)
                for kp in range(KP):
                    ksl = slice(2 * kp, 2 * kp + 2)
                    nc.tensor.matmul(
                        pj[:], w_sb[:, ksl, hsl], x8t[:, ksl, :],
                        start=(kp == 0), stop=(kp == KP - 1), perf_mode=DR,
                        skip_group_check=True)
                return pj

            def norm_part1(pj):
                """Evacuate proj PSUM (fp16 cast) + elementwise square."""
                ts_ = work.tile([128, TTS], f16, tag="tsn", bufs=4,
                                name="tsn")
                nc.vector.tensor_copy(ts_[:], pj[:])
                sq = work.tile([128, TTS], f16, tag="sqn", bufs=4,
                               name="sqn")
                nc.vector.tensor_mul(sq[:], ts_[:], ts_[:])
                return ts_, sq

            def norm_nsq(sq, tag_sfx):
                """Partition-sum of squares + 1/sqrt row (ACT Ln/Exp)."""
                nsq = psum.tile([1, TTS], f32, name=f"nsq{tag_sfx}",
                                tag="sp", bufs=2)
                nc.tensor.matmul(nsq[:], ones_red[:], sq[:])
                return nsq

            def norm_rows(nsq, is_k):
                lnr = rows.tile([1, TTS], f32, tag="lnr", bufs=3)
                nc.scalar.activation(lnr[:], nsq[:], AF.Ln)
                rq16 = rows.tile([1, TTS], f16, tag="rq16", bufs=3)
                ln_bias = ln_scale_c[:] if is_k else 0.0
                nc.scalar.activation(rq16[:], lnr[:], AF.Exp,
                                     scale=-0.5, bias=ln_bias)
                return rq16

            def norm_apply(ts_, rq16, dst_ap, tag_sfx):
                rqb = psum.tile([128, TTS], f32, name=f"rqb{tag_sfx}",
                                tag="sp", bufs=2)
                nc.tensor.matmul(rqb[:], ones_col[:], rq16[:])
                nc.vector.tensor_mul(dst_ap, ts_[:], rqb[:])

            def emit_qproj(tt, hh, x8t):
                """q projection + normalize for (token tile tt, head hh)."""
                if tt not in qn_tiles:
                    qn_tiles[tt] = qnpool.tile([128, HPC, TTS], f16,
                                               tag="qnt", bufs=3, name="qnt")
                pj = emit_proj8(wq8_sb, x8t, hh, f"pq_{tt}_{hh}")
                ts_, sq = norm_part1(pj)
                nsq = norm_nsq(sq, f"q{tt}{hh}")
                rq16 = norm_rows(nsq, False)
                norm_apply(ts_, rq16, qn_tiles[tt][:, hh, :], f"q{tt}{hh}")

            # ================= Stage A: K + V projections ==================
            for tt in range(NTT):
                tsl = slice(tt * TTS, (tt + 1) * TTS)
                if tt + 1 < NTT:
                    # prefetch the NEXT tile's x slices (4 chunks each so
                    # they spread across DMA queues) while this tile runs
                    nsl = slice((tt + 1) * TTS, (tt + 2) * TTS)
                    x8a[tt + 1] = x8pool.tile([128, KC, TTS], f8, tag="x8a",
                                              bufs=2, name="x8a")
                    xv[tt + 1] = xvpool.tile([128, KC, TTS], f16, tag="xv",
                                             bufs=2, name="xv")
                    for kh in range(4):
                        ks = slice(kh * 4, (kh + 1) * 4)
                        nc.sync.dma_start(x8a[tt + 1][:, ks, :],
                                          x8_t[:, ks, nsl])
                        nc.sync.dma_start(xv[tt + 1][:, ks, :],
                                          xT_t[:, ks, nsl])


                kparts = []
                for hh in range(HPC):
                    pk = emit_proj8(wk8_sb, x8a[tt], hh, f"pk_{tt}_{hh}")
                    kparts.append(norm_part1(pk))

                # V for both heads, natural layout (token on partitions).
                # The k-norm aux matmuls (nsq reductions, rqb broadcasts)
                # are spliced around the V blocks so the PE never waits on
                # the DVE square / ACT Ln-Exp latency of the norm chain.
                krows = [None, None]

                def v_block(sp2):
                    vp = psum.tile([128, 2 * DH], f32, name=f"vp_{tt}_{sp2}",
                                   tag="sp", bufs=2)
                    for half in range(2):
                        st = sp2 * 2 + half
                        for kc in range(KC):
                            nc.tensor.matmul(
                                vp[:, half * DH:(half + 1) * DH],
                                xv[tt][:, kc, st * 128:(st + 1) * 128],
                                wv_sb[:, kc, :], start=(kc == 0),
                                stop=(kc == KC - 1))
                    jidx = tt * NST + sp2 * 2
                    nc.vector.tensor_copy(vsb[:, jidx:jidx + 2, :], vp[:])

                if tt == 1:
                    nc.sync.dma_start(wo_sb[:], wo_r)
                nsq0 = norm_nsq(kparts[0][1], f"k{tt}0")
                krows[0] = norm_rows(nsq0, True)
                v_block(0)
                nsq1 = norm_nsq(kparts[1][1], f"k{tt}1")
                krows[1] = norm_rows(nsq1, True)
                v_block(1)
                for hh in range(HPC):
                    norm_apply(kparts[hh][0], krows[hh],
                               knt[:, hh, tsl], f"k{tt}{hh}")

                if tt == 0:
                    # q projection for tile 0 here, while x8a[0] is still
                    # resident (deferring it would hold the x8a buffer slot
                    # hostage across the whole stage-A rotation)
                    for hh in range(HPC):
                        emit_qproj(0, hh, x8a[0])

            # ============ Stage B: attention + stripes + out proj ==========
            x8b = {}
            for tt in range(NTT):
                tsl = slice(tt * TTS, (tt + 1) * TTS)
                qnt = qn_tiles[tt]
                if tt + 1 < NTT:
                    # prefetch x8 for the next tile's q stripe
                    x8b[tt + 1] = x8pool.tile([128, KC, TTS], f8, tag="x8b",
                                              bufs=2, name="x8b")
                    nc.sync.dma_start(
                        x8b[tt + 1][:], x8_t[:, :, slice((tt + 1) * TTS,
                                                         (tt + 2) * TTS)])
                ot_sb = [None, None]
                for h in range(HPC):
                    ot = psum.tile([128, TTS], f32, name=f"ot_{tt}_{h}",
                                   tag="ot", bufs=2)
                    acc = work.tile([128, TTS], f16, tag="acc", bufs=3)
                    NJP = NJC // 2           # 2-chunk pairs
                    e_tiles = {}

                    def st_pair(jp):
                        stp = psum.tile([128, 2 * TTS], f32,
                                        name=f"st_{tt}_{h}_{jp}",
                                        tag="st", bufs=2)
                        for jh in range(2):
                            jc = jp * 2 + jh
                            nc.tensor.matmul(
                                stp[:, jh * TTS:(jh + 1) * TTS],
                                knt[:, h, jc * 128:(jc + 1) * 128],
                                qnt[:, h, :], start=True, stop=True)
                        return stp

                    def exp_pair(jp, stp):
                        jq, half = jp // 2, jp % 2
                        if half == 0:
                            e_tiles[jq] = work.tile([128, 4 * TTS], f16,
                                                    tag="e", bufs=4, name="e")
                        e = e_tiles[jq]
                        esl = slice(half * 2 * TTS, (half + 1) * 2 * TTS)
                        if with_mask:
                            jc0 = jp * 2
                            mc = work.tile([128, 2, TTS], f16, tag="mc", bufs=3)
                            nc.sync.dma_start(
                                mc[:],
                                mT_d[:].rearrange("(c p) t -> p c t", p=128)
                                [:, jc0:jc0 + 2, tsl])
                            sm = work.tile([128, 2 * TTS], f32, tag="sm", bufs=3)
                            nc.vector.tensor_add(sm[:], stp[:], mc[:])
                            nc.scalar.activation(e[:, esl], sm[:], AF.Exp)
                        else:
                            nc.scalar.activation(e[:, esl], stp[:], AF.Exp)

                    def ot_pair(jp):
                        e = e_tiles[jp // 2]
                        for jh in range(2):
                            jc = jp * 2 + jh
                            lsl = slice((jp % 2 * 2 + jh) * TTS,
                                        (jp % 2 * 2 + jh + 1) * TTS)
                            nc.tensor.matmul(
                                ot[:], vsb[:, jc, h * 128:(h + 1) * 128],
                                e[:, lsl], start=(jc == 0),
                                stop=(jc == NJC - 1), skip_group_check=True)

                    def tree(jq):
                        # all-fp16 pair tree on DVE (fp16 SBUF adds hit the
                        # 2x perf mode, ~335ns; a GPSIMD offload was tried
                        # and inflated both engines via SBUF port conflicts)
                        e = e_tiles.pop(jq)
                        t01 = work.tile([128, 2 * TTS], f16, tag="t0", bufs=3)
                        nc.vector.tensor_add(t01[:], e[:, 0:2 * TTS],
                                             e[:, 2 * TTS:4 * TTS])
                        if jq == 0:
                            nc.vector.tensor_add(acc[:], t01[:, 0:TTS],
                                                 t01[:, TTS:2 * TTS])
                        else:
                            t2 = work.tile([128, TTS], f16, tag="t2", bufs=3)
                            nc.vector.tensor_add(t2[:], t01[:, 0:TTS],
                                                 t01[:, TTS:2 * TTS])
                            nc.vector.tensor_add(acc[:], acc[:], t2[:])

                    # q-proj stripe for the next tile: the DoubleRow
                    # matmuls are dripped INTO the jp loop (one after every
                    # odd OT pair) as independent PE filler; the accumulator
                    # lives in the "ot" psum tag alongside this head's OT.
                    stripe_pj = None
                    if tt + 1 < NTT:
                        if tt + 1 not in qn_tiles:
                            qn_tiles[tt + 1] = qnpool.tile(
                                [128, HPC, TTS], f16, tag="qnt", bufs=3,
                                name="qnt")
                        stripe_pj = psum.tile([128, TTS], f32,
                                              name=f"pq_{tt + 1}_{h}",
                                              tag="ot", bufs=2)
                        hsl = slice(h * HD, (h + 1) * HD)

                    def stripe_mm(kp):
                        if stripe_pj is None:
                            return
                        ksl = slice(2 * kp, 2 * kp + 2)
                        nc.tensor.matmul(
                            stripe_pj[:], wq8_sb[:, ksl, hsl],
                            x8b[tt + 1][:, ksl, :],
                            start=(kp == 0), stop=(kp == KP - 1),
                            perf_mode=DR, skip_group_check=True)

                    # software pipeline, depth 2: OT(jp) issues only after
                    # exp(jp) AND two newer ST pairs, so the PE never stalls
                    # on the ACT exp latency.
                    stps = [st_pair(0), st_pair(1)]
                    stripe_k = 0
                    for jp in range(NJP):
                        exp_pair(jp, stps[jp % 2])
                        if jp + 2 < NJP:
                            stps[jp % 2] = st_pair(jp + 2)
                        if jp % 2 == 1 and stripe_k < KP - 2:
                            # filler BEFORE the OT pair: the OT is the one
                            # waiting on this jp's exp, so the stripe matmul
                            # buys it ~240ns of semaphore-settling time
                            stripe_mm(stripe_k)
                            stripe_k += 1
                        ot_pair(jp)
                        if jp % 2 == 1:
                            tree(jp // 2)
                    while stripe_k < KP:
                        stripe_mm(stripe_k)
                        stripe_k += 1

                    # stripe epilogue part 1: evacuate + square (DVE)
                    if stripe_pj is not None:
                        s_ts, s_sq = norm_part1(stripe_pj)

                    # denominator -> 1/Z = Exp(-Ln(Z)) on ACT; stripe aux
                    # matmuls interleave so the PE has independent work
                    # while ACT runs the Ln/Exp chains.
                    z = psum.tile([1, TTS], f32, name=f"z_{tt}_{h}",
                                  tag="sp", bufs=2)
                    nc.tensor.matmul(z[:], ones_red[:], acc[:])
                    lnz = rows.tile([1, TTS], f32, tag="lnz", bufs=3)
                    nc.scalar.activation(lnz[:], z[:], AF.Ln)
                    rs16 = rows.tile([1, TTS], f16, tag="rs16", bufs=3)
                    nc.scalar.activation(rs16[:], lnz[:], AF.Exp, scale=-1.0)

                    if stripe_pj is not None:
                        s_nsq = norm_nsq(s_sq, f"q{tt + 1}{h}")
                        s_rq16 = norm_rows(s_nsq, False)

                    rsb = psum.tile([128, TTS], f32, name=f"rsb_{tt}_{h}",
                                    tag="sp", bufs=2)
                    nc.tensor.matmul(rsb[:], ones_col[:], rs16[:])
                    rsbs = work.tile([128, TTS], f32, tag="rsbs", bufs=2)
                    nc.vector.tensor_copy(rsbs[:], rsb[:])
                    osb = work.tile([128, TTS], f16, tag=f"osb{h}", bufs=2)
                    nc.vector.tensor_mul(osb[:], ot[:], rsbs[:])
                    ot_sb[h] = osb

                    if stripe_pj is not None:
                        norm_apply(s_ts, s_rq16,
                                   qn_tiles[tt + 1][:, h, :], f"q{tt + 1}{h}")

                # output projection: single-bank psum tiles in the sp tag so
                # this overlaps the next tile's attention.
                for st in range(NST):
                    for ng in range(D // 1024):
                        ops = []
                        for half in range(2):
                            nt = ng * 2 + half
                            ops.append(psum.tile(
                                [128, 512], f32, name=f"op_{tt}_{st}_{nt}",
                                tag="sp", bufs=2))
                        for h in range(HPC):
                            for half in range(2):
                                nt = ng * 2 + half
                                nc.tensor.matmul(
                                    ops[half][:],
                                    ot_sb[h][:, st * 128:(st + 1) * 128],
                                    wo_sb[:, h, nt * 512:(nt + 1) * 512],
                                    start=(h == 0), stop=(h == HPC - 1),
                                    skip_group_check=True)
                        for half in range(2):
                            nt = ng * 2 + half
                            oc = work.tile([128, 512], f32, tag="oc", bufs=4)
                            nc.vector.tensor_copy(oc[:], ops[half][:])
                            nc.sync.dma_start(
                                y_d[tt * TTS + st * 128:
                                    tt * TTS + (st + 1) * 128,
                                    nt * 512:(nt + 1) * 512], oc[:])

    return nc


def _get_program(t=T, with_mask=False):
    key = (t, with_mask)
    if key not in _PROG_CACHE:
        _PROG_CACHE[key] = build_program(t, with_mask)
    return _PROG_CACHE[key]


def _make_in_maps(x, attn_mask, W_qkv, W_out, use_mask):
    import concourse.mybir as mybir

    np8 = mybir.dt.np(mybir.dt.float8e4)
    xT = np.ascontiguousarray(x.T)
    xT16 = xT.astype(np.float16)
    x8 = xT.astype(np8)
    wq_f = W_qkv[:, 0 * D:1 * D]
    wk_f = W_qkv[:, 1 * D:2 * D]
    wv_f = W_qkv[:, 2 * D:3 * D]
    maskT = None
    if use_mask:
        maskT = np.ascontiguousarray(attn_mask.T).astype(np.float16)
    in_maps = []
    for c in range(NCORES):
        cs = slice(c * DH, (c + 1) * DH)
        m = {
            "xT": xT16,
            "x8": x8,
            "wq8": np.ascontiguousarray(wq_f[:, cs] * W8SCALE).astype(np8),
            "wk8": np.ascontiguousarray(wk_f[:, cs] * W8SCALE).astype(np8),
            "wv": np.ascontiguousarray(wv_f[:, cs]).astype(np.float16),
            "wo": np.ascontiguousarray(W_out[cs, :]).astype(np.float16),
        }
        if use_mask:
            m["maskT"] = maskT
        in_maps.append(m)
    return in_maps


def run_raw(x, attn_mask, W_qkv, W_out, trace=False, **kwargs):
    """Run the SPMD kernel; returns (full_output, BassKernelResults)."""
    from concourse.bass_utils import run_bass_kernel_spmd

    x = np.asarray(x, dtype=np.float32)
    attn_mask = np.asarray(attn_mask, dtype=np.float32)
    W_qkv = np.asarray(W_qkv, dtype=np.float32)
    W_out = np.asarray(W_out, dtype=np.float32)

    use_mask = bool(np.any(attn_mask))
    nc = _get_program(x.shape[0], use_mask)
    in_maps = _make_in_maps(x, attn_mask, W_qkv, W_out, use_mask)
    res = run_bass_kernel_spmd(nc, in_maps, core_ids=list(range(NCORES)),
                               trace=trace, **kwargs)
    out = np.zeros((x.shape[0], D), np.float32)
    for r in res.results:
        out += r["y"]
    return out, res


def kernel(x, attn_mask, W_qkv, W_out):
    out, _ = run_raw(x, attn_mask, W_qkv, W_out)
    return out
